# revision 1
# baseline (speedup 1.0000x reference)
"""Bass/Trainium2 kernel for nn_KeypointPPF_EdgeConv.

Strategy (8 NeuronCores, data-parallel over batch B=8):
  Host (numpy): fold BatchNorms into affine weights; compute PPF features and
  the tiny stage-A MLPs (pos_encoder, ppf layer1) on host; pre-transpose the
  big tensors into [ch, edge] tile layout so the device does only:
    e1:  psum1 = Wnf@nfT + Wposh@poshT + Wcd@kptT(bcast over k)   (PE, bf16)
    y1  = relu(psum1)                                             (ACT -> bf16)
    e2:  psum2 = W2@y1                                            (PE, bf16)
    out = reduce_max over k                                       (DVE)
  final relu(out + b2) on ACT; host transposes [256,4096] -> [4096,256].

Edge order: group g = 32 points x 16 neighbors (pt-major: f = pt*16 + k),
128 groups of 512 edges per core.
"""

import sys

sys.path.insert(0, "/opt/trn_rl_repo")

import numpy as np
import ml_dtypes

import concourse.bass as bass
import concourse.bacc as bacc
import concourse.mybir as mybir
import concourse.tile as tile
from concourse.bass_utils import run_bass_kernel_spmd

B, N, K, C, COUT = 8, 4096, 16, 128, 256
G = 128          # groups per core
PTS = 32         # points per group
F = PTS * K      # 512 edges per group
BN_EPS = 1e-5
BF16 = mybir.dt.bfloat16
F32 = mybir.dt.float32
NPBF16 = ml_dtypes.bfloat16

_CACHE = {}


def build_nc():
    nc = bacc.Bacc("TRN2", target_bir_lowering=False, debug=False)
    nfT = nc.declare_dram_parameter("nfT", [G, C, F], BF16, isOutput=False)
    poshT = nc.declare_dram_parameter("poshT", [G, 97, F], BF16, isOutput=False)
    kptT = nc.declare_dram_parameter("kptT", [C, N], BF16, isOutput=False)
    w_nf = nc.declare_dram_parameter("w_nf", [C, COUT], BF16, isOutput=False)
    w_cd = nc.declare_dram_parameter("w_cd", [C, COUT], BF16, isOutput=False)
    w_posh = nc.declare_dram_parameter("w_posh", [97, COUT], BF16, isOutput=False)
    w_e2a = nc.declare_dram_parameter("w_e2a", [128, COUT], BF16, isOutput=False)
    w_e2b = nc.declare_dram_parameter("w_e2b", [128, COUT], BF16, isOutput=False)
    bias2 = nc.declare_dram_parameter("bias2", [128, 2], F32, isOutput=False)
    out = nc.declare_dram_parameter("out", [COUT, N], F32, isOutput=True)

    with tile.TileContext(nc) as tc:
        with (
            tc.tile_pool(name="consts", bufs=1) as cpool,
            tc.tile_pool(name="loads", bufs=3) as lpool,
            tc.tile_pool(name="y1", bufs=3) as ypool,
            tc.tile_pool(name="outT", bufs=1) as opool,
            tc.tile_pool(name="psum", bufs=2, space="PSUM") as ppool,
        ):
            # resident constants
            kptT_sb = cpool.tile([C, N], BF16, tag="kptT")
            nc.sync.dma_start(kptT_sb[:], kptT[:])
            wnf_sb = cpool.tile([C, COUT], BF16, tag="wnf")
            nc.sync.dma_start(wnf_sb[:], w_nf[:])
            wcd_sb = cpool.tile([C, COUT], BF16, tag="wcd")
            nc.sync.dma_start(wcd_sb[:], w_cd[:])
            wposh_sb = cpool.tile([97, COUT], BF16, tag="wposh")
            nc.sync.dma_start(wposh_sb[:], w_posh[:])
            we2a_sb = cpool.tile([128, COUT], BF16, tag="we2a")
            nc.sync.dma_start(we2a_sb[:], w_e2a[:])
            we2b_sb = cpool.tile([128, COUT], BF16, tag="we2b")
            nc.sync.dma_start(we2b_sb[:], w_e2b[:])
            b2_sb = cpool.tile([128, 2], F32, tag="b2")
            nc.sync.dma_start(b2_sb[:], bias2[:])

            outT0 = opool.tile([128, N], F32, tag="outT0")
            outT1 = opool.tile([128, N], F32, tag="outT1")
            outTs = [outT0, outT1]

            for g in range(G):
                nfT_sb = lpool.tile([C, F], BF16, tag="nfT")
                # 1-elem memset absorbs the WAR wait on the Pool engine so the
                # DMA itself carries <=1 sync wait (walrus DIRECT2D limit)
                nc.gpsimd.memset(nfT_sb[0:1, 0:1], 0)
                nc.gpsimd.dma_start(nfT_sb[:], nfT[g])
                poshT_sb = lpool.tile([97, F], BF16, tag="poshT")
                nc.gpsimd.memset(poshT_sb[0:1, 0:1], 0)
                nc.gpsimd.dma_start(poshT_sb[:], poshT[g])

                # center rhs: [128, 32 pts] broadcast x16 over k (0-step AP)
                cd_rhs = (
                    kptT_sb[:, g * PTS:(g + 1) * PTS]
                    .unsqueeze(2)
                    .broadcast_to((C, PTS, K))
                )

                y1s = []
                for m in range(2):
                    mm = slice(m * 128, (m + 1) * 128)
                    psum1 = ppool.tile([128, F], F32, tag=f"psum1_{m}")
                    nc.tensor.matmul(
                        psum1[:], wnf_sb[:, mm], nfT_sb[:], start=True, stop=False
                    )
                    nc.tensor.matmul(
                        psum1[:], wcd_sb[:, mm], cd_rhs, start=False, stop=False
                    )
                    nc.tensor.matmul(
                        psum1[:], wposh_sb[:, mm], poshT_sb[:], start=False, stop=True
                    )
                    y1 = ypool.tile([128, F], BF16, tag=f"y1_{m}")
                    nc.scalar.activation(
                        y1[:], psum1[:], mybir.ActivationFunctionType.Relu
                    )
                    y1s.append(y1)

                for m in range(2):
                    mm = slice(m * 128, (m + 1) * 128)
                    psum2 = ppool.tile([128, F], F32, tag=f"psum2_{m}")
                    nc.tensor.matmul(
                        psum2[:], we2a_sb[:, mm], y1s[0][:], start=True, stop=False
                    )
                    nc.tensor.matmul(
                        psum2[:], we2b_sb[:, mm], y1s[1][:], start=False, stop=True
                    )
                    nc.vector.tensor_reduce(
                        outTs[m][:, g * PTS:(g + 1) * PTS],
                        psum2[:].rearrange("p (a b) -> p a b", b=K),
                        axis=mybir.AxisListType.X,
                        op=mybir.AluOpType.max,
                    )

            # final: relu(outT + b2) per channel, then store [256, N]
            for m in range(2):
                nc.scalar.activation(
                    outTs[m][:],
                    outTs[m][:],
                    mybir.ActivationFunctionType.Relu,
                    bias=b2_sb[:, m:m + 1],
                )
                nc.sync.dma_start(out[m * 128:(m + 1) * 128, :], outTs[m][:])
    nc.compile()
    return nc


def _prep(inputs):
    f32 = np.float32
    e1_w = inputs["e1_w"].astype(f32)
    s1 = inputs["e1_g"] / np.sqrt(inputs["e1_v"] + BN_EPS)
    t1 = inputs["e1_beta"] - inputs["e1_m"] * s1
    s2 = inputs["e2_g"] / np.sqrt(inputs["e2_v"] + BN_EPS)
    t2 = inputs["e2_beta"] - inputs["e2_m"] * s2
    sp = inputs["pos_g"] / np.sqrt(inputs["pos_v"] + BN_EPS)
    tp = inputs["pos_beta"] - inputs["pos_m"] * sp
    sf = inputs["ppf_g"] / np.sqrt(inputs["ppf_v"] + BN_EPS)
    tf = inputs["ppf_beta"] - inputs["ppf_m"] * sf

    W_c, W_d = e1_w[:, 0:128], e1_w[:, 128:256]
    W_p, W_q = e1_w[:, 256:320], e1_w[:, 320:384]

    A_nf = s1[:, None] * W_d                         # [256,128]
    A_cd = s1[:, None] * (W_c - W_d)                 # [256,128]
    A_pos = s1[:, None] * W_q                        # [256,64]
    A_h = (s1[:, None] * W_p) @ inputs["ppf_w2"]     # [256,32]
    b1p = s1 * (inputs["e1_b"] + W_p @ inputs["ppf_b2"]) + t1
    A_posh = np.concatenate([A_pos, A_h, b1p[:, None]], axis=1)  # [256,97]
    W2p = s2[:, None] * inputs["e2_w"]
    b2p = s2 * inputs["e2_b"] + t2

    # host stage-A features
    kx = inputs["kpt_xyz"]                            # [B,N,3]
    nx = inputs["neighbor_xyz"]                       # [B,N,K,3]
    nn = inputs["neighbor_normals"]
    rel = nx - kx[:, :, None, :]
    kn = nn.mean(axis=2)
    kn = kn / np.maximum(np.linalg.norm(kn, axis=-1, keepdims=True), 1e-12)
    n1 = kn[:, :, None, :]
    d_norm = np.linalg.norm(rel, axis=-1, keepdims=True)
    d = rel / (d_norm + 1e-8)
    alpha = np.clip(np.sum(n1 * d, -1, keepdims=True), -1.0, 1.0)
    phi = np.clip(np.sum(nn * d, -1, keepdims=True), -1.0, 1.0)
    theta = np.clip(np.sum(n1 * nn, -1, keepdims=True), -1.0, 1.0)
    ppf = np.concatenate([d_norm, alpha, phi, theta], -1)  # [B,N,K,4]

    Wpe = (inputs["pos_w"] * sp[:, None]).T           # [3,64]
    cpe = sp * inputs["pos_b"] + tp
    W1e = (inputs["ppf_w1"] * sf[:, None]).T          # [4,32]
    c1e = sf * inputs["ppf_b1"] + tf
    pos_enc = np.maximum(rel @ Wpe + cpe, 0.0)        # [B,N,K,64]
    h = np.maximum(ppf @ W1e + c1e, 0.0)              # [B,N,K,32]
    posh = np.concatenate(
        [pos_enc, h, np.ones(h.shape[:3] + (1,), f32)], axis=-1
    ).astype(f32)                                     # [B,N,K,97]

    weights = {
        "w_nf": np.ascontiguousarray(A_nf.T).astype(NPBF16),
        "w_cd": np.ascontiguousarray(A_cd.T).astype(NPBF16),
        "w_posh": np.ascontiguousarray(A_posh.T).astype(NPBF16),
        "w_e2a": np.ascontiguousarray(W2p.T[0:128]).astype(NPBF16),
        "w_e2b": np.ascontiguousarray(W2p.T[128:256]).astype(NPBF16),
        "bias2": np.ascontiguousarray(
            b2p.astype(f32).reshape(2, 128).T
        ),                                            # [128,2] col m = chunk m
    }

    in_maps = []
    for b in range(B):
        # [N,K,C] -> groups [G, 512, C] -> [G, C, 512]
        nf_g = (
            inputs["neighbor_feature"][b]
            .reshape(G, F, C)
            .transpose(0, 2, 1)
        )
        posh_g = posh[b].reshape(G, F, 97).transpose(0, 2, 1)
        m = {
            "nfT": np.ascontiguousarray(nf_g).astype(NPBF16),
            "poshT": np.ascontiguousarray(posh_g).astype(NPBF16),
            "kptT": np.ascontiguousarray(inputs["kpt_feature"][b].T).astype(
                NPBF16
            ),
        }
        m.update(weights)
        in_maps.append(m)
    return in_maps


def kernel(trace=False, **inputs):
    if "nc" not in _CACHE:
        _CACHE["nc"] = build_nc()
    nc = _CACHE["nc"]
    in_maps = _prep(inputs)
    res = run_bass_kernel_spmd(nc, in_maps, list(range(B)), trace=trace)
    out = np.stack([res.results[b]["out"].T for b in range(B)])  # [B,N,COUT]
    _CACHE["last"] = res
    return np.ascontiguousarray(out.astype(np.float32))



# revision 2
# speedup vs baseline: 6869.9288x; 6869.9288x over previous
"""Bass/Trainium2 kernel for nn_KeypointPPF_EdgeConv (optimized).

Data-parallel over batch B=8 (one NeuronCore per batch element).

Per group g (32 points x 16 neighbors = 512 edges), all bf16 on PE:
  e1 psum[m] = Wnf[m] @ nfT(g)  +  Wposhcd[g,m] @ [poshT(g); onehot]
  y1[m] = relu(psum[m] + b1p[m])          (ACT, per-partition bias)
  e2 psum[m] = We2a[m] @ y1[0] + We2b[m] @ y1[1]
  outT[:, pts] = max over k               (DVE reduce)
  every 16 groups: outT = max(outT + b2p, 0) (DVE) and DMA out.

Key structure:
- The center term A_cd @ kpt (per-point, broadcast over k) is injected into
  the spare 32 contraction rows of the posh matmul: rhs rows 96:128 hold a
  static one-hot point selector, weight rows 96:128 hold per-group cd values
  (cd = kpt @ A_cd.T on host) — an exact rank-32 factorization that removes
  a whole 512-column PE stream per output chunk (8 instead of 10 streams).
- Software pipelining (1-pair skew) over PAIRS of groups: PE streams
  e1(pair p+1) while ACT converts psum1(pair p); within a pair the two
  groups' matmuls sharing a weight set run back-to-back, so the PE loads
  nf/e2 weights once per pair (10 LDWEIGHTS per 2 groups instead of 16).
- DMA issue cost dominates the Pool engine (~500 ns per dma_start), so
  inputs are packed 4 groups per transfer host-side: 3 dma_starts per 4
  groups instead of 4 per group.
"""

import sys

sys.path.insert(0, "/opt/trn_rl_repo")

import numpy as np
import ml_dtypes

import concourse.bass as bass
import concourse.bacc as bacc
import concourse.mybir as mybir
import concourse.tile as tile
from concourse.bass_utils import run_bass_kernel_spmd

B, N, K, C, COUT = 8, 4096, 16, 128, 256
G = 128          # groups per core
PTS = 32         # points per group
F = PTS * K      # 512 edges per group
SG = 4           # groups per DMA super-group
NSG = G // SG
BN_EPS = 1e-5
BF16 = mybir.dt.bfloat16
F32 = mybir.dt.float32
NPBF16 = ml_dtypes.bfloat16

_CACHE = {}


def build_nc():
    nc = bacc.Bacc("TRN2", target_bir_lowering=False, debug=False)
    nfT = nc.declare_dram_parameter("nfT", [NSG, C, SG * F], BF16, isOutput=False)
    poshT = nc.declare_dram_parameter("poshT", [NSG, 96, SG * F], BF16, isOutput=False)
    cdw = nc.declare_dram_parameter("cdw", [NSG, PTS, SG * 256], BF16, isOutput=False)
    onehot = nc.declare_dram_parameter("onehot", [PTS, SG * F], BF16, isOutput=False)
    w_nf = nc.declare_dram_parameter("w_nf", [C, COUT], BF16, isOutput=False)
    w_posh = nc.declare_dram_parameter("w_posh", [96, SG * 256], BF16, isOutput=False)
    w_e2a = nc.declare_dram_parameter("w_e2a", [128, COUT], BF16, isOutput=False)
    w_e2b = nc.declare_dram_parameter("w_e2b", [128, COUT], BF16, isOutput=False)
    biases = nc.declare_dram_parameter("biases", [128, 4], F32, isOutput=False)
    out = nc.declare_dram_parameter("out", [COUT, N], F32, isOutput=True)

    SLICES = 16                  # groups per output flush
    NS = G // SLICES

    with tile.TileContext(nc) as tc:
        with (
            tc.tile_pool(name="consts", bufs=1) as cpool,
            tc.tile_pool(name="loads", bufs=3) as lpool,
            tc.tile_pool(name="wposh", bufs=1) as wpool,
            tc.tile_pool(name="y1", bufs=3) as ypool,
            tc.tile_pool(name="outT", bufs=1) as opool,
            tc.tile_pool(name="psum", bufs=2, space="PSUM") as ppool,
        ):
            # resident constants
            wnf_sb = cpool.tile([C, COUT], BF16, tag="wnf")
            nc.sync.dma_start(wnf_sb[:], w_nf[:])
            we2a_sb = cpool.tile([128, COUT], BF16, tag="we2a")
            nc.sync.dma_start(we2a_sb[:], w_e2a[:])
            we2b_sb = cpool.tile([128, COUT], BF16, tag="we2b")
            nc.sync.dma_start(we2b_sb[:], w_e2b[:])
            b_sb = cpool.tile([128, 4], F32, tag="biases")
            nc.sync.dma_start(b_sb[:], biases[:])

            # Manually rotated persistent buffers (one per super-group lap):
            # posh rhs rows 96:128 = static one-hot; stitched weight rows
            # 0:96 = static A_posh (replicated per 256-col group block).
            NBUF = 3
            posh_bufs = []
            wposh_bufs = []
            for i in range(NBUF):
                t = wpool.tile([128, SG * F], BF16, tag=f"posh_{i}",
                               name=f"posh_{i}")
                nc.sync.dma_start(t[96:128, :], onehot[:])
                posh_bufs.append(t)
                w = wpool.tile([128, SG * 256], BF16, tag=f"wposh_{i}",
                               name=f"wposh_{i}")
                nc.sync.dma_start(w[0:96, :], w_posh[:])
                wposh_bufs.append(w)

            def emit_dma(sg):
                """One DMA bundle covering SG consecutive groups."""
                nfT_sb = lpool.tile([C, SG * F], BF16, tag="nfT",
                                    name=f"nfT_{sg}")
                # 1-elem memset absorbs the WAR wait on the Pool engine so
                # the DMA itself carries <=1 sync wait (walrus DIRECT2D)
                nc.gpsimd.memset(nfT_sb[0:1, 0:1], 0)
                nc.gpsimd.dma_start(nfT_sb[:], nfT[sg])
                posh_sb = posh_bufs[sg % NBUF]
                nc.gpsimd.memset(posh_sb[0:1, 0:1], 0)
                nc.gpsimd.dma_start(posh_sb[0:96, :], poshT[sg])
                w = wposh_bufs[sg % NBUF]
                nc.gpsimd.memset(w[96:97, 0:1], 0)
                nc.gpsimd.dma_start(w[96:128, :], cdw[sg])
                return nfT_sb

            def emit_e1(pair, nfT_sbs):
                gs = [2 * pair, 2 * pair + 1]
                psums = {g: [None, None] for g in gs}
                for m in range(2):
                    mm = slice(m * 128, (m + 1) * 128)
                    for g in gs:
                        sgi = g % SG
                        fsl = slice(sgi * F, (sgi + 1) * F)
                        psum1 = ppool.tile([128, F], F32, tag=f"psum1_{m}",
                                           name=f"psum1_{m}_{g}")
                        nc.tensor.matmul(
                            psum1[:], wnf_sb[:, mm],
                            nfT_sbs[g // SG][:, fsl],
                            start=True, stop=False,
                        )
                        psums[g][m] = psum1
                    for g in gs:
                        sgi = g % SG
                        wsl = slice(sgi * 256 + m * 128,
                                    sgi * 256 + (m + 1) * 128)
                        fsl = slice(sgi * F, (sgi + 1) * F)
                        nc.tensor.matmul(
                            psums[g][m][:],
                            wposh_bufs[(g // SG) % NBUF][:, wsl],
                            posh_bufs[(g // SG) % NBUF][:, fsl],
                            start=False, stop=True,
                        )
                return psums

            def emit_act(pair, psums):
                gs = [2 * pair, 2 * pair + 1]
                y1s = {g: [None, None] for g in gs}
                for m in range(2):
                    for g in gs:
                        y1 = ypool.tile([128, F], BF16, tag=f"y1_{m}",
                                        name=f"y1_{m}_{g}")
                        nc.scalar.activation(
                            y1[:], psums[g][m][:],
                            mybir.ActivationFunctionType.Relu,
                            bias=b_sb[:, m:m + 1],
                        )
                        y1s[g][m] = y1
                return y1s

            def emit_e2(pair, y1s, outTs):
                gs = [2 * pair, 2 * pair + 1]
                for m in range(2):
                    mm = slice(m * 128, (m + 1) * 128)
                    psum2s = {}
                    for g in gs:
                        psum2 = ppool.tile([128, F], F32, tag=f"psum2_{m}",
                                           name=f"psum2_{m}_{g}")
                        nc.tensor.matmul(
                            psum2[:], we2a_sb[:, mm], y1s[g][0][:],
                            start=True, stop=False,
                        )
                        psum2s[g] = psum2
                    for g in gs:
                        nc.tensor.matmul(
                            psum2s[g][:], we2b_sb[:, mm], y1s[g][1][:],
                            start=False, stop=True,
                        )
                    for g in gs:
                        gi = g % SLICES
                        nc.vector.tensor_reduce(
                            outTs[m][:, gi * PTS:(gi + 1) * PTS],
                            psum2s[g][:].rearrange("p (a b) -> p a b", b=K),
                            axis=mybir.AxisListType.X,
                            op=mybir.AluOpType.max,
                        )

            def make_outTs(s):
                return [
                    opool.tile([128, SLICES * PTS], F32, tag=f"outT_{s}_{m}",
                               name=f"outT_{s}_{m}")
                    for m in range(2)
                ]

            def flush(s, outTs):
                for m in range(2):
                    nc.vector.tensor_scalar(
                        outTs[m][:], outTs[m][:],
                        scalar1=b_sb[:, 2 + m:3 + m], scalar2=0.0,
                        op0=mybir.AluOpType.add, op1=mybir.AluOpType.max,
                    )
                    nc.sync.dma_start(
                        out[m * 128:(m + 1) * 128,
                            s * SLICES * PTS:(s + 1) * SLICES * PTS],
                        outTs[m][:],
                    )

            # prologue: prefetch 2 super-groups, start e1(pair 0)
            NP = G // 2
            nfs = {0: emit_dma(0), 1: emit_dma(1)}
            psums = {0: emit_e1(0, nfs)}
            outTs = make_outTs(0)
            for p in range(NP):
                if p % 2 == 0 and p // 2 + 2 < NSG:
                    nfs[p // 2 + 2] = emit_dma(p // 2 + 2)
                y1s = emit_act(p, psums.pop(p))
                if p + 1 < NP:
                    psums[p + 1] = emit_e1(p + 1, nfs)
                emit_e2(p, y1s, outTs)
                if (2 * p + 1) % SLICES == SLICES - 1:
                    flush((2 * p + 1) // SLICES, outTs)
                    if p + 1 < NP:
                        outTs = make_outTs((2 * p + 1) // SLICES + 1)
    nc.compile()
    return nc


def _prep(inputs):
    f32 = np.float32
    e1_w = inputs["e1_w"].astype(f32)
    s1 = inputs["e1_g"] / np.sqrt(inputs["e1_v"] + BN_EPS)
    t1 = inputs["e1_beta"] - inputs["e1_m"] * s1
    s2 = inputs["e2_g"] / np.sqrt(inputs["e2_v"] + BN_EPS)
    t2 = inputs["e2_beta"] - inputs["e2_m"] * s2
    sp = inputs["pos_g"] / np.sqrt(inputs["pos_v"] + BN_EPS)
    tp = inputs["pos_beta"] - inputs["pos_m"] * sp
    sf = inputs["ppf_g"] / np.sqrt(inputs["ppf_v"] + BN_EPS)
    tf = inputs["ppf_beta"] - inputs["ppf_m"] * sf

    W_c, W_d = e1_w[:, 0:128], e1_w[:, 128:256]
    W_p, W_q = e1_w[:, 256:320], e1_w[:, 320:384]

    A_nf = s1[:, None] * W_d                         # [256,128]
    A_cd = s1[:, None] * (W_c - W_d)                 # [256,128]
    A_pos = s1[:, None] * W_q                        # [256,64]
    A_h = (s1[:, None] * W_p) @ inputs["ppf_w2"]     # [256,32]
    b1p = s1 * (inputs["e1_b"] + W_p @ inputs["ppf_b2"]) + t1
    A_posh = np.concatenate([A_pos, A_h], axis=1)    # [256,96]
    W2p = s2[:, None] * inputs["e2_w"]
    b2p = s2 * inputs["e2_b"] + t2

    # host stage-A features
    kx = inputs["kpt_xyz"]                            # [B,N,3]
    nx = inputs["neighbor_xyz"]                       # [B,N,K,3]
    nn = inputs["neighbor_normals"]
    rel = nx - kx[:, :, None, :]
    kn = nn.mean(axis=2)
    kn = kn / np.maximum(np.linalg.norm(kn, axis=-1, keepdims=True), 1e-12)
    n1 = kn[:, :, None, :]
    d_norm = np.linalg.norm(rel, axis=-1, keepdims=True)
    d = rel / (d_norm + 1e-8)
    alpha = np.clip(np.sum(n1 * d, -1, keepdims=True), -1.0, 1.0)
    phi = np.clip(np.sum(nn * d, -1, keepdims=True), -1.0, 1.0)
    theta = np.clip(np.sum(n1 * nn, -1, keepdims=True), -1.0, 1.0)
    ppf = np.concatenate([d_norm, alpha, phi, theta], -1)  # [B,N,K,4]

    Wpe = (inputs["pos_w"] * sp[:, None]).T           # [3,64]
    cpe = sp * inputs["pos_b"] + tp
    W1e = (inputs["ppf_w1"] * sf[:, None]).T          # [4,32]
    c1e = sf * inputs["ppf_b1"] + tf
    pos_enc = np.maximum(rel @ Wpe + cpe, 0.0)        # [B,N,K,64]
    h = np.maximum(ppf @ W1e + c1e, 0.0)              # [B,N,K,32]
    posh = np.concatenate([pos_enc, h], axis=-1).astype(f32)  # [B,N,K,96]

    onehot1 = np.kron(np.eye(PTS, dtype=f32), np.ones((1, K), f32))  # [32,512]

    weights = {
        "w_nf": np.ascontiguousarray(A_nf.T).astype(NPBF16),
        "w_posh": np.ascontiguousarray(np.tile(A_posh.T, (1, SG))).astype(NPBF16),
        "w_e2a": np.ascontiguousarray(W2p.T[0:128]).astype(NPBF16),
        "w_e2b": np.ascontiguousarray(W2p.T[128:256]).astype(NPBF16),
        "onehot": np.ascontiguousarray(np.tile(onehot1, (1, SG))).astype(NPBF16),
        "biases": np.ascontiguousarray(
            np.stack([b1p[0:128], b1p[128:256], b2p[0:128], b2p[128:256]], axis=1)
        ).astype(f32),                                # [128,4]
    }

    in_maps = []
    for b in range(B):
        nf_g = (
            inputs["neighbor_feature"][b]
            .reshape(NSG, SG * F, C)
            .transpose(0, 2, 1)
        )
        posh_g = posh[b].reshape(NSG, SG * F, 96).transpose(0, 2, 1)
        cd = inputs["kpt_feature"][b].astype(f32) @ A_cd.T        # [N,256]
        cdw = (
            cd.reshape(NSG, SG, PTS, 256)
            .transpose(0, 2, 1, 3)
            .reshape(NSG, PTS, SG * 256)
        )
        m = {
            "nfT": np.ascontiguousarray(nf_g).astype(NPBF16),
            "poshT": np.ascontiguousarray(posh_g).astype(NPBF16),
            "cdw": np.ascontiguousarray(cdw).astype(NPBF16),
        }
        m.update(weights)
        in_maps.append(m)
    return in_maps


def kernel(trace=False, **inputs):
    if "nc" not in _CACHE:
        _CACHE["nc"] = build_nc()
    nc = _CACHE["nc"]
    in_maps = _prep(inputs)
    res = run_bass_kernel_spmd(nc, in_maps, list(range(B)), trace=trace)
    out = np.stack([res.results[b]["out"].T for b in range(B)])  # [B,N,COUT]
    _CACHE["last"] = res
    return np.ascontiguousarray(out.astype(np.float32))


# revision 3
# speedup vs baseline: 8374.1398x; 1.2190x over previous
"""Bass/Trainium2 kernel for nn_KeypointPPF_EdgeConv (optimized).

Data-parallel over batch B=8 (one NeuronCore per batch element).

Per group g (32 points x 16 neighbors = 512 edges), all bf16 on PE:
  e1 psum[m] = Wnf[m] @ nfT(g)  +  Wposhcd[g,m] @ [poshT(g); onehot]
  y1[m] = relu(psum[m] + b1p[m])          (ACT, per-partition bias)
  e2 psum[m] = We2a[m] @ y1[0] + We2b[m] @ y1[1]
  outT[:, pts] = max over k               (DVE reduce)
  every 16 groups: outT = max(outT + b2p, 0) (DVE) and DMA out.

Key structure:
- The center term A_cd @ kpt (per-point, broadcast over k) is injected into
  the spare 32 contraction rows of the posh matmul: rhs rows 96:128 hold a
  static one-hot point selector, weight rows 96:128 hold per-group cd values
  (cd = kpt @ A_cd.T on host) — an exact rank-32 factorization that removes
  a whole 512-column PE stream per output chunk (8 instead of 10 streams).
- Software pipelining (1-pair skew) over PAIRS of groups: PE streams
  e1(pair p+1) while ACT converts psum1(pair p); within a pair the two
  groups' matmuls sharing a weight set run back-to-back, so the PE loads
  nf/e2 weights once per pair (10 LDWEIGHTS per 2 groups instead of 16).
- DMA issue cost dominates the Pool engine (~500 ns per dma_start), so
  inputs are packed 4 groups per transfer host-side: 3 dma_starts per 4
  groups instead of 4 per group.
"""

import sys

sys.path.insert(0, "/opt/trn_rl_repo")

import numpy as np
import ml_dtypes

import concourse.bass as bass
import concourse.bacc as bacc
import concourse.mybir as mybir
import concourse.tile as tile
from concourse.bass_utils import run_bass_kernel_spmd

B, N, K, C, COUT = 8, 4096, 16, 128, 256
G = 128          # groups per core
PTS = 32         # points per group
F = PTS * K      # 512 edges per group
SG = 4           # groups per DMA super-group
NSG = G // SG
BN_EPS = 1e-5
BF16 = mybir.dt.bfloat16
F32 = mybir.dt.float32
NPBF16 = ml_dtypes.bfloat16

_CACHE = {}


def build_nc():
    nc = bacc.Bacc("TRN2", target_bir_lowering=False, debug=False)
    nfT = nc.declare_dram_parameter("nfT", [NSG, C, SG * F], BF16, isOutput=False)
    poshT = nc.declare_dram_parameter("poshT", [NSG, 96, SG * F], BF16, isOutput=False)
    cdw = nc.declare_dram_parameter("cdw", [NSG, PTS, SG * 256], BF16, isOutput=False)
    onehot = nc.declare_dram_parameter("onehot", [PTS, SG * F], BF16, isOutput=False)
    w_nf = nc.declare_dram_parameter("w_nf", [C, COUT], BF16, isOutput=False)
    w_posh = nc.declare_dram_parameter("w_posh", [96, SG * 256], BF16, isOutput=False)
    w_e2a = nc.declare_dram_parameter("w_e2a", [128, COUT], BF16, isOutput=False)
    w_e2b = nc.declare_dram_parameter("w_e2b", [128, COUT], BF16, isOutput=False)
    biases = nc.declare_dram_parameter("biases", [128, 4], F32, isOutput=False)
    out = nc.declare_dram_parameter("out", [COUT, N], F32, isOutput=True)

    SLICES = 16                  # groups per output flush
    NS = G // SLICES

    with tile.TileContext(nc) as tc:
        with (
            tc.tile_pool(name="consts", bufs=1) as cpool,
            tc.tile_pool(name="loads", bufs=3) as lpool,
            tc.tile_pool(name="wposh", bufs=1) as wpool,
            tc.tile_pool(name="y1", bufs=3) as ypool,
            tc.tile_pool(name="outT", bufs=1) as opool,
            tc.tile_pool(name="psum", bufs=2, space="PSUM") as ppool,
        ):
            # resident constants
            wnf_sb = cpool.tile([C, COUT], BF16, tag="wnf")
            nc.sync.dma_start(wnf_sb[:], w_nf[:])
            we2a_sb = cpool.tile([128, COUT], BF16, tag="we2a")
            nc.sync.dma_start(we2a_sb[:], w_e2a[:])
            we2b_sb = cpool.tile([128, COUT], BF16, tag="we2b")
            nc.sync.dma_start(we2b_sb[:], w_e2b[:])
            b_sb = cpool.tile([128, 4], F32, tag="biases")
            nc.sync.dma_start(b_sb[:], biases[:])

            # Manually rotated persistent buffers (one per super-group lap):
            # posh rhs rows 96:128 = static one-hot; stitched weight rows
            # 0:96 = static A_posh (replicated per 256-col group block).
            NBUF = 3
            posh_bufs = []
            wposh_bufs = []
            for i in range(NBUF):
                t = wpool.tile([128, SG * F], BF16, tag=f"posh_{i}",
                               name=f"posh_{i}")
                nc.sync.dma_start(t[96:128, :], onehot[:])
                posh_bufs.append(t)
                w = wpool.tile([128, SG * 256], BF16, tag=f"wposh_{i}",
                               name=f"wposh_{i}")
                nc.sync.dma_start(w[0:96, :], w_posh[:])
                wposh_bufs.append(w)

            def emit_dma(sg):
                """One DMA bundle covering SG consecutive groups."""
                nfT_sb = lpool.tile([C, SG * F], BF16, tag="nfT",
                                    name=f"nfT_{sg}")
                # 1-elem memset absorbs the WAR wait on the Pool engine so
                # the DMA itself carries <=1 sync wait (walrus DIRECT2D)
                nc.gpsimd.memset(nfT_sb[0:1, 0:1], 0)
                nc.gpsimd.dma_start(nfT_sb[:], nfT[sg])
                posh_sb = posh_bufs[sg % NBUF]
                nc.gpsimd.memset(posh_sb[0:1, 0:1], 0)
                nc.gpsimd.dma_start(posh_sb[0:96, :], poshT[sg])
                w = wposh_bufs[sg % NBUF]
                nc.gpsimd.memset(w[96:97, 0:1], 0)
                nc.gpsimd.dma_start(w[96:128, :], cdw[sg])
                return nfT_sb

            def emit_e1(pair, nfT_sbs):
                gs = [2 * pair, 2 * pair + 1]
                psums = {g: [None, None] for g in gs}
                for m in range(2):
                    mm = slice(m * 128, (m + 1) * 128)
                    for g in gs:
                        sgi = g % SG
                        fsl = slice(sgi * F, (sgi + 1) * F)
                        psum1 = ppool.tile([128, F], F32, tag=f"psum1_{m}",
                                           name=f"psum1_{m}_{g}")
                        nc.tensor.matmul(
                            psum1[:], wnf_sb[:, mm],
                            nfT_sbs[g // SG][:, fsl],
                            start=True, stop=False,
                        )
                        psums[g][m] = psum1
                    for g in gs:
                        sgi = g % SG
                        wsl = slice(sgi * 256 + m * 128,
                                    sgi * 256 + (m + 1) * 128)
                        fsl = slice(sgi * F, (sgi + 1) * F)
                        nc.tensor.matmul(
                            psums[g][m][:],
                            wposh_bufs[(g // SG) % NBUF][:, wsl],
                            posh_bufs[(g // SG) % NBUF][:, fsl],
                            start=False, stop=True,
                        )
                return psums

            def emit_act(pair, psums):
                gs = [2 * pair, 2 * pair + 1]
                y1s = {g: [None, None] for g in gs}
                for m in range(2):
                    for g in gs:
                        y1 = ypool.tile([128, F], BF16, tag=f"y1_{m}",
                                        name=f"y1_{m}_{g}")
                        nc.scalar.activation(
                            y1[:], psums[g][m][:],
                            mybir.ActivationFunctionType.Relu,
                            bias=b_sb[:, m:m + 1],
                        )
                        y1s[g][m] = y1
                return y1s

            def emit_e2(pair, y1s, outTs):
                gs = [2 * pair, 2 * pair + 1]
                for m in range(2):
                    mm = slice(m * 128, (m + 1) * 128)
                    psum2s = {}
                    for g in gs:
                        psum2 = ppool.tile([128, F], F32, tag=f"psum2_{m}",
                                           name=f"psum2_{m}_{g}")
                        nc.tensor.matmul(
                            psum2[:], we2a_sb[:, mm], y1s[g][0][:],
                            start=True, stop=False,
                        )
                        psum2s[g] = psum2
                    for g in gs:
                        nc.tensor.matmul(
                            psum2s[g][:], we2b_sb[:, mm], y1s[g][1][:],
                            start=False, stop=True,
                        )
                    for g in gs:
                        gi = g % SLICES
                        nc.vector.tensor_reduce(
                            outTs[m][:, gi * PTS:(gi + 1) * PTS],
                            psum2s[g][:].rearrange("p (a b) -> p a b", b=K),
                            axis=mybir.AxisListType.X,
                            op=mybir.AluOpType.max,
                        )

            def make_outTs(s):
                return [
                    opool.tile([128, SLICES * PTS], F32, tag=f"outT_{s}_{m}",
                               name=f"outT_{s}_{m}")
                    for m in range(2)
                ]

            def flush(s, outTs):
                for m in range(2):
                    nc.vector.tensor_scalar(
                        outTs[m][:], outTs[m][:],
                        scalar1=b_sb[:, 2 + m:3 + m], scalar2=0.0,
                        op0=mybir.AluOpType.add, op1=mybir.AluOpType.max,
                    )
                    nc.sync.dma_start(
                        out[m * 128:(m + 1) * 128,
                            s * SLICES * PTS:(s + 1) * SLICES * PTS],
                        outTs[m][:],
                    )

            # prologue: prefetch 2 super-groups, start e1(pair 0)
            NP = G // 2
            nfs = {0: emit_dma(0), 1: emit_dma(1)}
            psums = {0: emit_e1(0, nfs)}
            outTs = make_outTs(0)
            for p in range(NP):
                if p % 2 == 0 and p // 2 + 2 < NSG:
                    nfs[p // 2 + 2] = emit_dma(p // 2 + 2)
                y1s = emit_act(p, psums.pop(p))
                if p + 1 < NP:
                    psums[p + 1] = emit_e1(p + 1, nfs)
                emit_e2(p, y1s, outTs)
                if (2 * p + 1) % SLICES == SLICES - 1:
                    flush((2 * p + 1) // SLICES, outTs)
                    if p + 1 < NP:
                        outTs = make_outTs((2 * p + 1) // SLICES + 1)
    nc.compile()
    return nc


def _prep(inputs):
    f32 = np.float32
    e1_w = inputs["e1_w"].astype(f32)
    s1 = inputs["e1_g"] / np.sqrt(inputs["e1_v"] + BN_EPS)
    t1 = inputs["e1_beta"] - inputs["e1_m"] * s1
    s2 = inputs["e2_g"] / np.sqrt(inputs["e2_v"] + BN_EPS)
    t2 = inputs["e2_beta"] - inputs["e2_m"] * s2
    sp = inputs["pos_g"] / np.sqrt(inputs["pos_v"] + BN_EPS)
    tp = inputs["pos_beta"] - inputs["pos_m"] * sp
    sf = inputs["ppf_g"] / np.sqrt(inputs["ppf_v"] + BN_EPS)
    tf = inputs["ppf_beta"] - inputs["ppf_m"] * sf

    W_c, W_d = e1_w[:, 0:128], e1_w[:, 128:256]
    W_p, W_q = e1_w[:, 256:320], e1_w[:, 320:384]

    A_nf = s1[:, None] * W_d                         # [256,128]
    A_cd = s1[:, None] * (W_c - W_d)                 # [256,128]
    A_pos = s1[:, None] * W_q                        # [256,64]
    A_h = (s1[:, None] * W_p) @ inputs["ppf_w2"]     # [256,32]
    b1p = s1 * (inputs["e1_b"] + W_p @ inputs["ppf_b2"]) + t1
    A_posh = np.concatenate([A_pos, A_h], axis=1)    # [256,96]
    W2p = s2[:, None] * inputs["e2_w"]
    b2p = s2 * inputs["e2_b"] + t2

    # host stage-A features
    kx = inputs["kpt_xyz"]                            # [B,N,3]
    nx = inputs["neighbor_xyz"]                       # [B,N,K,3]
    nn = inputs["neighbor_normals"]
    rel = nx - kx[:, :, None, :]
    kn = nn.mean(axis=2)
    kn = kn / np.maximum(np.linalg.norm(kn, axis=-1, keepdims=True), 1e-12)
    n1 = kn[:, :, None, :]
    d_norm = np.linalg.norm(rel, axis=-1, keepdims=True)
    d = rel / (d_norm + 1e-8)
    alpha = np.clip(np.sum(n1 * d, -1, keepdims=True), -1.0, 1.0)
    phi = np.clip(np.sum(nn * d, -1, keepdims=True), -1.0, 1.0)
    theta = np.clip(np.sum(n1 * nn, -1, keepdims=True), -1.0, 1.0)
    ppf = np.concatenate([d_norm, alpha, phi, theta], -1)  # [B,N,K,4]

    Wpe = (inputs["pos_w"] * sp[:, None]).T           # [3,64]
    cpe = sp * inputs["pos_b"] + tp
    W1e = (inputs["ppf_w1"] * sf[:, None]).T          # [4,32]
    c1e = sf * inputs["ppf_b1"] + tf
    pos_enc = np.maximum(rel @ Wpe + cpe, 0.0)        # [B,N,K,64]
    h = np.maximum(ppf @ W1e + c1e, 0.0)              # [B,N,K,32]
    posh = np.concatenate([pos_enc, h], axis=-1).astype(f32)  # [B,N,K,96]

    onehot1 = np.kron(np.eye(PTS, dtype=f32), np.ones((1, K), f32))  # [32,512]

    weights = {
        "w_nf": np.ascontiguousarray(A_nf.T).astype(NPBF16),
        "w_posh": np.ascontiguousarray(np.tile(A_posh.T, (1, SG))).astype(NPBF16),
        "w_e2a": np.ascontiguousarray(W2p.T[0:128]).astype(NPBF16),
        "w_e2b": np.ascontiguousarray(W2p.T[128:256]).astype(NPBF16),
        "onehot": np.ascontiguousarray(np.tile(onehot1, (1, SG))).astype(NPBF16),
        "biases": np.ascontiguousarray(
            np.stack([b1p[0:128], b1p[128:256], b2p[0:128], b2p[128:256]], axis=1)
        ).astype(f32),                                # [128,4]
    }

    in_maps = []
    for b in range(B):
        nf_g = (
            inputs["neighbor_feature"][b]
            .reshape(NSG, SG * F, C)
            .transpose(0, 2, 1)
        )
        posh_g = posh[b].reshape(NSG, SG * F, 96).transpose(0, 2, 1)
        cd = inputs["kpt_feature"][b].astype(f32) @ A_cd.T        # [N,256]
        cdw = (
            cd.reshape(NSG, SG, PTS, 256)
            .transpose(0, 2, 1, 3)
            .reshape(NSG, PTS, SG * 256)
        )
        m = {
            "nfT": np.ascontiguousarray(nf_g).astype(NPBF16),
            "poshT": np.ascontiguousarray(posh_g).astype(NPBF16),
            "cdw": np.ascontiguousarray(cdw).astype(NPBF16),
        }
        m.update(weights)
        in_maps.append(m)
    return in_maps


def kernel(trace=False, **inputs):
    inputs = {k: np.asarray(v) for k, v in inputs.items()}
    if "nc" not in _CACHE:
        _CACHE["nc"] = build_nc()
    nc = _CACHE["nc"]
    in_maps = _prep(inputs)
    res = run_bass_kernel_spmd(nc, in_maps, list(range(B)), trace=trace)
    out = np.stack([res.results[b]["out"].T for b in range(B)])  # [B,N,COUT]
    _CACHE["last"] = res
    return np.ascontiguousarray(out.astype(np.float32))


# revision 4
# speedup vs baseline: 9158.4473x; 1.0937x over previous
"""Bass/Trainium2 kernel for nn_KeypointPPF_EdgeConv (optimized).

Data-parallel over batch B=8 (one NeuronCore per batch element).

Per group g (32 points x 16 neighbors = 512 edges), all bf16 on PE:
  e1 psum[m] = Wnf[m] @ nfT(g)  +  Wposhcd[g,m] @ [poshT(g); onehot]
  y1[m] = relu(psum[m] + b1p[m])          (ACT, per-partition bias)
  e2 psum[m] = We2a[m] @ y1[0] + We2b[m] @ y1[1]
  outT[:, pts] = max over k               (DVE reduce)
  every 16 groups: outT = max(outT + b2p, 0) (DVE) and DMA out.

Key structure:
- The center term A_cd @ kpt (per-point, broadcast over k) is injected into
  the spare 32 contraction rows of the posh matmul: rhs rows 96:128 hold a
  static one-hot point selector, weight rows 96:128 hold per-group cd values
  (cd = kpt @ A_cd.T on host) — an exact rank-32 factorization that removes
  a whole 512-column PE stream per output chunk (8 instead of 10 streams).
- Software pipelining (1-pair skew) over PAIRS of groups: PE streams
  e1(pair p+1) while ACT converts psum1(pair p); within a pair the two
  groups' matmuls sharing a weight set run back-to-back, so the PE loads
  nf/e2 weights once per pair (10 LDWEIGHTS per 2 groups instead of 16).
- DMA issue cost dominates the Pool engine (~500 ns per dma_start), so
  inputs are packed 4 groups per transfer host-side: 3 dma_starts per 4
  groups instead of 4 per group.
"""

import sys

sys.path.insert(0, "/opt/trn_rl_repo")

import numpy as np
import ml_dtypes

import concourse.bass as bass
import concourse.bacc as bacc
import concourse.mybir as mybir
import concourse.tile as tile
from concourse.bass_utils import run_bass_kernel_spmd

B, N, K, C, COUT = 8, 4096, 16, 128, 256
G = 128          # groups per core
PTS = 32         # points per group
F = PTS * K      # 512 edges per group
SG = 4           # groups per DMA super-group
NSG = G // SG
BN_EPS = 1e-5
BF16 = mybir.dt.bfloat16
F32 = mybir.dt.float32
NPBF16 = ml_dtypes.bfloat16

_CACHE = {}


def build_nc():
    nc = bacc.Bacc("TRN2", target_bir_lowering=False, debug=False)
    nfT = nc.declare_dram_parameter("nfT", [NSG, C, SG * F], BF16, isOutput=False)
    poshT = nc.declare_dram_parameter("poshT", [NSG, 96, SG * F], BF16, isOutput=False)
    cdw = nc.declare_dram_parameter("cdw", [NSG, PTS, SG * 256], BF16, isOutput=False)
    onehot = nc.declare_dram_parameter("onehot", [PTS, SG * F], BF16, isOutput=False)
    w_nf = nc.declare_dram_parameter("w_nf", [C, COUT], BF16, isOutput=False)
    w_posh = nc.declare_dram_parameter("w_posh", [96, SG * 256], BF16, isOutput=False)
    w_e2a = nc.declare_dram_parameter("w_e2a", [128, COUT], BF16, isOutput=False)
    w_e2b = nc.declare_dram_parameter("w_e2b", [128, COUT], BF16, isOutput=False)
    biases = nc.declare_dram_parameter("biases", [128, 4], F32, isOutput=False)
    out = nc.declare_dram_parameter("out", [COUT, N], F32, isOutput=True)

    SLICES = 16                  # groups per output flush
    NS = G // SLICES

    with tile.TileContext(nc) as tc:
        with (
            tc.tile_pool(name="consts", bufs=1) as cpool,
            tc.tile_pool(name="loads", bufs=3) as lpool,
            tc.tile_pool(name="wposh", bufs=1) as wpool,
            tc.tile_pool(name="y1", bufs=3) as ypool,
            tc.tile_pool(name="outT", bufs=1) as opool,
            tc.tile_pool(name="psum", bufs=2, space="PSUM") as ppool,
        ):
            # resident constants
            wnf_sb = cpool.tile([C, COUT], BF16, tag="wnf")
            nc.sync.dma_start(wnf_sb[:], w_nf[:])
            we2a_sb = cpool.tile([128, COUT], BF16, tag="we2a")
            nc.sync.dma_start(we2a_sb[:], w_e2a[:])
            we2b_sb = cpool.tile([128, COUT], BF16, tag="we2b")
            nc.sync.dma_start(we2b_sb[:], w_e2b[:])
            b_sb = cpool.tile([128, 4], F32, tag="biases")
            nc.sync.dma_start(b_sb[:], biases[:])

            # Manually rotated persistent buffers (one per super-group lap):
            # posh rhs rows 96:128 = static one-hot; stitched weight rows
            # 0:96 = static A_posh (replicated per 256-col group block).
            NBUF = 3
            posh_bufs = []
            wposh_bufs = []
            for i in range(NBUF):
                t = wpool.tile([128, SG * F], BF16, tag=f"posh_{i}",
                               name=f"posh_{i}")
                nc.sync.dma_start(t[96:128, :], onehot[:])
                posh_bufs.append(t)
                w = wpool.tile([128, SG * 256], BF16, tag=f"wposh_{i}",
                               name=f"wposh_{i}")
                nc.sync.dma_start(w[0:96, :], w_posh[:])
                wposh_bufs.append(w)

            def emit_dma(sg):
                """One DMA bundle covering SG consecutive groups."""
                nfT_sb = lpool.tile([C, SG * F], BF16, tag="nfT",
                                    name=f"nfT_{sg}")
                # 1-elem memset absorbs the WAR wait on the Pool engine so
                # the DMA itself carries <=1 sync wait (walrus DIRECT2D)
                nc.gpsimd.memset(nfT_sb[0:1, 0:1], 0)
                nc.gpsimd.dma_start(nfT_sb[:], nfT[sg])
                posh_sb = posh_bufs[sg % NBUF]
                nc.gpsimd.memset(posh_sb[0:1, 0:1], 0)
                nc.gpsimd.dma_start(posh_sb[0:96, :], poshT[sg])
                w = wposh_bufs[sg % NBUF]
                nc.gpsimd.memset(w[96:97, 0:1], 0)
                nc.gpsimd.dma_start(w[96:128, :], cdw[sg])
                return nfT_sb

            def emit_e1(pair, nfT_sbs):
                gs = [2 * pair, 2 * pair + 1]
                psums = {g: [None, None] for g in gs}
                for m in range(2):
                    mm = slice(m * 128, (m + 1) * 128)
                    for g in gs:
                        sgi = g % SG
                        fsl = slice(sgi * F, (sgi + 1) * F)
                        psum1 = ppool.tile([128, F], F32, tag=f"psum1_{m}",
                                           name=f"psum1_{m}_{g}")
                        nc.tensor.matmul(
                            psum1[:], wnf_sb[:, mm],
                            nfT_sbs[g // SG][:, fsl],
                            start=True, stop=False,
                        )
                        psums[g][m] = psum1
                    for g in gs:
                        sgi = g % SG
                        wsl = slice(sgi * 256 + m * 128,
                                    sgi * 256 + (m + 1) * 128)
                        fsl = slice(sgi * F, (sgi + 1) * F)
                        nc.tensor.matmul(
                            psums[g][m][:],
                            wposh_bufs[(g // SG) % NBUF][:, wsl],
                            posh_bufs[(g // SG) % NBUF][:, fsl],
                            start=False, stop=True,
                        )
                return psums

            def emit_act(pair, psums):
                gs = [2 * pair, 2 * pair + 1]
                y1s = {g: [None, None] for g in gs}
                for m in range(2):
                    for g in gs:
                        y1 = ypool.tile([128, F], BF16, tag=f"y1_{m}",
                                        name=f"y1_{m}_{g}")
                        nc.scalar.activation(
                            y1[:], psums[g][m][:],
                            mybir.ActivationFunctionType.Relu,
                            bias=b_sb[:, m:m + 1],
                        )
                        y1s[g][m] = y1
                return y1s

            def emit_e2(pair, y1s, outTs):
                gs = [2 * pair, 2 * pair + 1]
                for m in range(2):
                    mm = slice(m * 128, (m + 1) * 128)
                    psum2s = {}
                    for g in gs:
                        psum2 = ppool.tile([128, F], F32, tag=f"psum2_{m}",
                                           name=f"psum2_{m}_{g}")
                        nc.tensor.matmul(
                            psum2[:], we2a_sb[:, mm], y1s[g][0][:],
                            start=True, stop=False,
                        )
                        psum2s[g] = psum2
                    for g in gs:
                        nc.tensor.matmul(
                            psum2s[g][:], we2b_sb[:, mm], y1s[g][1][:],
                            start=False, stop=True,
                        )
                    for g in gs:
                        gi = g % SLICES
                        nc.vector.tensor_reduce(
                            outTs[m][:, gi * PTS:(gi + 1) * PTS],
                            psum2s[g][:].rearrange("p (a b) -> p a b", b=K),
                            axis=mybir.AxisListType.X,
                            op=mybir.AluOpType.max,
                        )

            def make_outTs(s):
                return [
                    opool.tile([128, SLICES * PTS], F32, tag=f"outT_{s}_{m}",
                               name=f"outT_{s}_{m}")
                    for m in range(2)
                ]

            def flush(s, outTs):
                for m in range(2):
                    nc.vector.tensor_scalar(
                        outTs[m][:], outTs[m][:],
                        scalar1=b_sb[:, 2 + m:3 + m], scalar2=0.0,
                        op0=mybir.AluOpType.add, op1=mybir.AluOpType.max,
                    )
                    nc.sync.dma_start(
                        out[m * 128:(m + 1) * 128,
                            s * SLICES * PTS:(s + 1) * SLICES * PTS],
                        outTs[m][:],
                    )

            # prologue: prefetch 2 super-groups, start e1(pair 0)
            NP = G // 2
            nfs = {0: emit_dma(0), 1: emit_dma(1)}
            psums = {0: emit_e1(0, nfs)}
            outTs = make_outTs(0)
            for p in range(NP):
                if p % 2 == 0 and p // 2 + 2 < NSG:
                    nfs[p // 2 + 2] = emit_dma(p // 2 + 2)
                y1s = emit_act(p, psums.pop(p))
                if p + 1 < NP:
                    psums[p + 1] = emit_e1(p + 1, nfs)
                emit_e2(p, y1s, outTs)
                if (2 * p + 1) % SLICES == SLICES - 1:
                    flush((2 * p + 1) // SLICES, outTs)
                    if p + 1 < NP:
                        outTs = make_outTs((2 * p + 1) // SLICES + 1)
    nc.compile()
    return nc


def _prep(inputs):
    f32 = np.float32
    e1_w = inputs["e1_w"].astype(f32)
    s1 = inputs["e1_g"] / np.sqrt(inputs["e1_v"] + BN_EPS)
    t1 = inputs["e1_beta"] - inputs["e1_m"] * s1
    s2 = inputs["e2_g"] / np.sqrt(inputs["e2_v"] + BN_EPS)
    t2 = inputs["e2_beta"] - inputs["e2_m"] * s2
    sp = inputs["pos_g"] / np.sqrt(inputs["pos_v"] + BN_EPS)
    tp = inputs["pos_beta"] - inputs["pos_m"] * sp
    sf = inputs["ppf_g"] / np.sqrt(inputs["ppf_v"] + BN_EPS)
    tf = inputs["ppf_beta"] - inputs["ppf_m"] * sf

    W_c, W_d = e1_w[:, 0:128], e1_w[:, 128:256]
    W_p, W_q = e1_w[:, 256:320], e1_w[:, 320:384]

    A_nf = s1[:, None] * W_d                         # [256,128]
    A_cd = s1[:, None] * (W_c - W_d)                 # [256,128]
    A_pos = s1[:, None] * W_q                        # [256,64]
    A_h = (s1[:, None] * W_p) @ inputs["ppf_w2"]     # [256,32]
    b1p = s1 * (inputs["e1_b"] + W_p @ inputs["ppf_b2"]) + t1
    A_posh = np.concatenate([A_pos, A_h], axis=1)    # [256,96]
    W2p = s2[:, None] * inputs["e2_w"]
    b2p = s2 * inputs["e2_b"] + t2

    # host stage-A features
    kx = inputs["kpt_xyz"]                            # [B,N,3]
    nx = inputs["neighbor_xyz"]                       # [B,N,K,3]
    nn = inputs["neighbor_normals"]
    rel = nx - kx[:, :, None, :]
    kn = nn.mean(axis=2)
    kn = kn / np.maximum(np.linalg.norm(kn, axis=-1, keepdims=True), 1e-12)
    n1 = kn[:, :, None, :]
    d_norm = np.linalg.norm(rel, axis=-1, keepdims=True)
    d = rel / (d_norm + 1e-8)
    alpha = np.clip(np.sum(n1 * d, -1, keepdims=True), -1.0, 1.0)
    phi = np.clip(np.sum(nn * d, -1, keepdims=True), -1.0, 1.0)
    theta = np.clip(np.sum(n1 * nn, -1, keepdims=True), -1.0, 1.0)
    ppf = np.concatenate([d_norm, alpha, phi, theta], -1)  # [B,N,K,4]

    Wpe = (inputs["pos_w"] * sp[:, None]).T           # [3,64]
    cpe = sp * inputs["pos_b"] + tp
    W1e = (inputs["ppf_w1"] * sf[:, None]).T          # [4,32]
    c1e = sf * inputs["ppf_b1"] + tf
    pos_enc = np.maximum(rel @ Wpe + cpe, 0.0)        # [B,N,K,64]
    h = np.maximum(ppf @ W1e + c1e, 0.0)              # [B,N,K,32]
    posh = np.concatenate([pos_enc, h], axis=-1).astype(f32)  # [B,N,K,96]

    onehot1 = np.kron(np.eye(PTS, dtype=f32), np.ones((1, K), f32))  # [32,512]

    weights = {
        "w_nf": np.ascontiguousarray(A_nf.T).astype(NPBF16),
        "w_posh": np.ascontiguousarray(np.tile(A_posh.T, (1, SG))).astype(NPBF16),
        "w_e2a": np.ascontiguousarray(W2p.T[0:128]).astype(NPBF16),
        "w_e2b": np.ascontiguousarray(W2p.T[128:256]).astype(NPBF16),
        "onehot": np.ascontiguousarray(np.tile(onehot1, (1, SG))).astype(NPBF16),
        "biases": np.ascontiguousarray(
            np.stack([b1p[0:128], b1p[128:256], b2p[0:128], b2p[128:256]], axis=1)
        ).astype(f32),                                # [128,4]
    }

    in_maps = []
    for b in range(B):
        nf_g = (
            inputs["neighbor_feature"][b]
            .reshape(NSG, SG * F, C)
            .transpose(0, 2, 1)
        )
        posh_g = posh[b].reshape(NSG, SG * F, 96).transpose(0, 2, 1)
        cd = inputs["kpt_feature"][b].astype(f32) @ A_cd.T        # [N,256]
        cdw = (
            cd.reshape(NSG, SG, PTS, 256)
            .transpose(0, 2, 1, 3)
            .reshape(NSG, PTS, SG * 256)
        )
        m = {
            "nfT": np.ascontiguousarray(nf_g).astype(NPBF16),
            "poshT": np.ascontiguousarray(posh_g).astype(NPBF16),
            "cdw": np.ascontiguousarray(cdw).astype(NPBF16),
        }
        m.update(weights)
        in_maps.append(m)
    return in_maps


def kernel(trace=False, **inputs):
    inputs = {k: np.asarray(v) for k, v in inputs.items()}
    if "nc" not in _CACHE:
        _CACHE["nc"] = build_nc()
    nc = _CACHE["nc"]
    in_maps = _prep(inputs)
    try:
        res = run_bass_kernel_spmd(nc, in_maps, list(range(B)), trace=trace)
    except Exception:
        # The axon-tunneled devices occasionally wedge with
        # NRT_EXEC_UNIT_UNRECOVERABLE; a single re-execution has always
        # recovered in practice.
        import time as _time
        _time.sleep(5.0)
        res = run_bass_kernel_spmd(nc, in_maps, list(range(B)), trace=trace)
    out = np.stack([res.results[b]["out"].T for b in range(B)])  # [B,N,COUT]
    _CACHE["last"] = res
    return np.ascontiguousarray(out.astype(np.float32))
